# revision 14
# baseline (speedup 1.0000x reference)
"""Causal self-attention (B=4, T=2048, C=1024, H=16) on 8 Trainium2 NeuronCores.

Core index = 2*batch + head_group: each core owns one batch element and 8 of
the 16 heads (tensor-parallel split of c_attn output dim / c_proj input dim).
Each core emits a partial projection out^T [C, T] in fp16; the host sums the
two head-group partials per batch and adds the bias terms.

fp16 datapath (fp32 PSUM accumulation, fp32 softmax denominator).

Schedule (single emission-ordered stream; Tile preserves per-engine order):
  B(tn) units: qkT[co-pair] = W_qk^T x^T (+bias, DVE); v = x @ W_v (ACT copy)
  C groups (ic, hp, jt): head-PAIR processing — heads 2hp (partitions 0:64)
     and 2hp+1 (64:128) issue row-tiled S^T matmuls back-to-back so they run
     CONCURRENTLY on the PE (K=64 each, tile_position (0,0)/(64,0) auto).
     One exp ACT covers both heads, trimmed to [lo:512] on diagonal blocks;
     causal masking via post-exp DVE multiply with a constant tri tile.
     PV matmuls of group g-1 are emitted after S of group g (software
     pipelining) so the PE never waits on the ACT exp.
  B(tn+1) and D projection units are interleaved into the C stream by an
     ACT-vs-PE deficit counter: the C stream alone is ACT-bound ~1.7x, so
     independent full-array matmuls fill the PE and keep HAM at K=8/8.
  D: out^T = W_p^T yT -> fp32 psum -> DVE fp16 copy -> DMA out.
"""

import numpy as np

import concourse.bass as bass
import concourse.mybir as mybir
import concourse.tile as tile
from concourse import bacc, bass_utils

B, T, C, H = 4, 2048, 1024, 16
HD = C // H          # 64 head dim
N_CORES = 8
HG = H // 2          # 8 heads per core
CL = HG * HD         # 512 local width of q/k/v
TT = T // 128        # 16 t-tiles
CB = C // 128        # 8 c-tiles
DB = CL // 128       # 4 local-hd tiles
NIC = T // 512       # i-chunks (4)

f32 = mybir.dt.float32
f16 = mybir.dt.float16

_PROG_CACHE = {}


def _emit(tc, aps):
    nc = tc.nc
    Exp = mybir.ActivationFunctionType.Exp
    Copy = mybir.ActivationFunctionType.Copy

    from contextlib import ExitStack

    with ExitStack() as outer:
        const = outer.enter_context(tc.tile_pool(name="const", bufs=1))
        p_xT = outer.enter_context(tc.tile_pool(name="xT", bufs=1))
        p_w = outer.enter_context(tc.tile_pool(name="wsb", bufs=1))
        p_qkT = outer.enter_context(tc.tile_pool(name="qkT", bufs=1))
        p_v = outer.enter_context(tc.tile_pool(name="vv", bufs=1))
        p_yT = outer.enter_context(tc.tile_pool(name="yT", bufs=1))
        p_pt = outer.enter_context(tc.tile_pool(name="pt", bufs=6))
        p_ot = outer.enter_context(tc.tile_pool(name="ot", bufs=3))
        p_r = outer.enter_context(tc.tile_pool(name="rpool", bufs=4))
        ps = outer.enter_context(tc.tile_pool(name="ps", bufs=3, space="PSUM"))
        ps_u = outer.enter_context(tc.tile_pool(name="psu", bufs=2, space="PSUM"))

        # ---- input DMAs: first-needed first; consts on the gpsimd queue ----
        # device co slots: [q01, k01, q23, k23] pairs -> hp0/hp1 unlock first
        wqk_sb = p_w.tile([128, CB, CB * 128], f16)  # [c-part, cb, co*128+...]
        def emit_wqk_dma(ci):
            nc.scalar.dma_start(
                wqk_sb[:, :, ci * 256 : (ci + 1) * 256],
                aps[f"wqk{ci}"].rearrange("(cb p) n -> p cb n", p=128),
            )
        emit_wqk_dma(0)
        emit_wqk_dma(1)
        xT = p_xT.tile([128, CB, T], f16)

        def emit_xT(tn):
            if tn > 0:  # WAW fence: delays the DMA until DVE reaches this point
                nc.vector.memset(xT[:, 0:1, tn * 512 : tn * 512 + 8], 0.0)
            nc.sync.dma_start_transpose(
                xT[:, :, tn * 512 : (tn + 1) * 512],
                aps["x"][tn * 512 : (tn + 1) * 512, :],
            )

        emit_xT(0)
        wv_sb = p_w.tile([128, CB, CL], f16)
        nc.scalar.dma_start(wv_sb[:], aps["wv"].rearrange("(cb p) n -> p cb n", p=128))
        emit_wqk_dma(2)
        emit_wqk_dma(3)
        wp_sb = p_w.tile([128, DB, C], f16)
        nc.scalar.dma_start(wp_sb[:], aps["wp"].rearrange("(db p) c -> p db c", p=128))
        negI = const.tile([128, 128], f16)  # -60000 * I
        nc.gpsimd.dma_start(negI[:], aps["negI"])
        mask2 = const.tile([128, 2, 128], f16)  # 1 where row > col (mask out)
        nc.gpsimd.dma_start(mask2[:], aps["mask2"])
        bqk = const.tile([128, CB], f32)
        nc.gpsimd.dma_start(bqk[:], aps["bqk"].rearrange("co p -> p co"))
        warm = const.tile([1, 8], f32)
        nc.scalar.activation(warm[0:1, 0:1], bqk[0:1, 0:1], Exp)  # preload exp table

        qkT = {
            (co, tn): p_qkT.tile([128, 512], f16, tag=f"qkT_{co}_{tn}", name=f"qkT_{co}_{tn}")
            for co in range(CB)
            for tn in range(NIC)
        }
        vv = {}
        for jt in range(TT):
            vv[jt] = p_v.tile([128, HG, HD + 1], f16, tag=f"vv_{jt}", name=f"vv_{jt}")
            nc.vector.memset(vv[jt][:, :, HD : HD + 1], 1.0)
        yTn = {tn: p_yT.tile([128, DB, 512], f16, tag=f"yT_{tn}", name=f"yT_{tn}") for tn in range(NIC)}

        # ------------- emission units -------------
        def emit_qk_unit(tn, co0):
            """qkT tiles for co0, co0+1 at i-chunk tn (16 MMs + 2 DVE adds)."""
            g = ps.tile([128, 2, 512], f32, tag="g", name="g")
            for ix in range(2):
                co = co0 + ix
                for cb in range(CB):
                    nc.tensor.matmul(
                        g[:, ix, :],
                        wqk_sb[:, cb, co * 128 : (co + 1) * 128],
                        xT[:, cb, tn * 512 : (tn + 1) * 512],
                        start=(cb == 0),
                        stop=(cb == CB - 1),
                    )
            for ix in range(2):
                co = co0 + ix
                nc.vector.tensor_scalar_add(
                    qkT[(co, tn)][:], g[:, ix, :], bqk[:, co : co + 1]
                )

        def emit_v_unit(tn, u):
            """vv tiles for t-tiles 4*tn+2u, +1 (16 MMs + 2 ACT copies)."""
            g = ps.tile([128, 2, 512], f32, tag="g", name="g")
            for ix in range(2):
                tt = 4 * tn + 2 * u + ix
                for cb in range(CB):
                    nc.tensor.matmul(
                        g[:, ix, :],
                        xT[:, cb, tt * 128 : (tt + 1) * 128],
                        wv_sb[:, cb, :],
                        start=(cb == 0),
                        stop=(cb == CB - 1),
                    )
            for ix in range(2):
                tt = 4 * tn + 2 * u + ix
                nc.vector.tensor_copy(
                    vv[tt][:, :, 0:HD],
                    g[:, ix, :].rearrange("p (h d) -> p h d", d=HD),
                )

        def emit_proj_unit(tn, co0):
            """out^T rows for co0, co0+1 at i-chunk tn (8 MMs + ACT copy + DMA)."""
            g = ps.tile([128, 2, 512], f32, tag="g", name="g")
            for ix in range(2):
                co = co0 + ix
                for db in range(DB):
                    nc.tensor.matmul(
                        g[:, ix, :],
                        wp_sb[:, db, co * 128 : (co + 1) * 128],
                        yTn[tn][:, db, :],
                        start=(db == 0),
                        stop=(db == DB - 1),
                    )
            ot = p_ot.tile([128, 2, 512], f16, tag="ot", name="ot")
            nc.vector.tensor_copy(ot[:], g[:])
            for ix in range(2):
                co = co0 + ix
                nc.sync.dma_start(
                    aps["outT"][co * 128 : (co + 1) * 128, tn * 512 : (tn + 1) * 512],
                    ot[:, ix, :],
                )

        def emit_normalize(hp, ic, u, poff):
            # two PSUM reads release the U accumulator slot; rest runs on SBUF
            usb = p_r.tile([HD, 512], f32, tag="usb", name="usb")
            nc.vector.tensor_copy(usb[:], u[0:HD, :])
            rs = p_r.tile([1, 512], f32, tag="rs", name="rs")
            nc.vector.tensor_copy(rs[:], u[HD : HD + 1, :])
            rr = p_r.tile([1, 512], f32, tag="rr", name="rr")
            nc.vector.reciprocal_approx_fast(rr[:], rs[:])
            rb = p_r.tile([HD, 512], f32, tag="rb", name="rb")
            nc.gpsimd.partition_broadcast(rb[:], rr[0:1, :], channels=HD)
            nc.gpsimd.tensor_mul(yTn[ic][poff : poff + HD, hp, :], usb[:], rb[:])

        CO_Q = (0, 1, 4, 5)
        CO_K = (2, 3, 6, 7)

        def emit_group(ic, hp, jt, uA, uB):
            """S^T for head pair (2hp, 2hp+1) at (jt, ic); returns PV closure."""
            co_q, co_k = CO_Q[hp], CO_K[hp]
            m = jt % 4
            diag = jt // 4 == ic
            lo = 128 * m if diag else 0
            kt = jt // 4
            g = ps.tile([128, 2, 512], f32, tag="g", name="g")
            nc.tensor.matmul(
                g[:, 0, lo:512],
                qkT[(co_k, kt)][0:64, m * 128 : (m + 1) * 128],
                qkT[(co_q, ic)][0:64, lo:512],
                start=True,
                stop=not diag,
                skip_group_check=True,
            )
            nc.tensor.matmul(
                g[:, 1, lo:512],
                qkT[(co_k, kt)][64:128, m * 128 : (m + 1) * 128],
                qkT[(co_q, ic)][64:128, lo:512],
                start=True,
                stop=not diag,
                skip_group_check=True,
            )
            if diag:  # -60000 above the block diagonal -> exp == 0
                nc.tensor.matmul(
                    g[:, 0:2, lo : lo + 128], negI[:], mask2[:],
                    start=False, stop=True, skip_group_check=True,
                )
            pt = p_pt.tile([128, 2, 512], f16, tag="pt", name="pt")
            nc.scalar.activation(
                pt[:, 0:2, lo:512], g[:, 0:2, lo:512], Exp, scale=1.0 / np.sqrt(HD)
            )

            def pv():
                nc.tensor.matmul(
                    uA[:, lo:512],
                    vv[jt][:, 2 * hp, :],
                    pt[:, 0, lo:512],
                    start=(jt == 0),
                    stop=(jt == 4 * ic + 3),
                )
                nc.tensor.matmul(
                    uB[:, lo:512],
                    vv[jt][:, 2 * hp + 1, :],
                    pt[:, 1, lo:512],
                    start=(jt == 0),
                    stop=(jt == 4 * ic + 3),
                )
                if jt == 4 * ic + 3:
                    emit_normalize(hp, ic, uA, 0)
                    emit_normalize(hp, ic, uB, 64)

            w = 512 - lo
            act_ns = (2 * w + 352) / 1.2 + 100
            pe_ns = 3 * w / 2.4 + 120 + (280 if diag else 0)
            return pv, act_ns - pe_ns

        # ------------- the schedule -------------
        # B(0): q01+k01 unlock C(0) hp0/hp1; q23/k23 go through the filler
        emit_qk_unit(0, 0)
        emit_qk_unit(0, 2)
        emit_v_unit(0, 0)
        emit_v_unit(0, 1)
        emit_xT(1)

        filler = []  # (pe_cost_ns, key, fn) in emission-feasible order
        state = {"deficit": 7500.0}
        pending = []  # PV closures, lag 2

        def run_pending(keep=0):
            while len(pending) > keep:
                pending.pop(0)()

        def pull_filler():
            while filler and state["deficit"] >= filler[0][0]:
                pe_cost, _, fn = filler.pop(0)
                fn()
                state["deficit"] -= pe_cost

        def flush_key(key):
            kept = []
            for item in filler:
                if item[1] == key:
                    item[2]()
                    state["deficit"] -= item[0]
                else:
                    kept.append(item)
            filler[:] = kept
            state["deficit"] = max(state["deficit"], -3000.0)

        filler.append((3600, ("q23", 0), lambda: emit_qk_unit(0, 4)))
        filler.append((3600, ("k23", 0), lambda: emit_qk_unit(0, 6)))
        for ic in range(NIC):
            if ic + 1 < NIC:
                tn = ic + 1
                filler.append((3600, ("q01", tn), lambda t=tn: emit_qk_unit(t, 0)))
                filler.append((3600, ("k01", tn), lambda t=tn: emit_qk_unit(t, 2)))
                filler.append((3600, ("q23", tn), lambda t=tn: emit_qk_unit(t, 4)))
                filler.append((3600, ("k23", tn), lambda t=tn: emit_qk_unit(t, 6)))
                filler.append((3600, ("v0", tn), lambda t=tn: emit_v_unit(t, 0)))
                filler.append((3600, ("v1", tn), lambda t=tn: emit_v_unit(t, 1)))
            for hp in range(4):
                if hp == 0:
                    flush_key(("q01", ic))
                if hp == 2:
                    flush_key(("q23", ic))
                uA = ps_u.tile([HD + 1, 512], f32, tag="u", name="uA")
                uB = ps_u.tile([HD + 1, 512], f32, tag="u", name="uB")
                for jt in range(4 * (ic + 1)):
                    if hp == 0 and jt == 4 * ic:
                        flush_key(("k01", ic))
                        flush_key(("v0", ic))
                    if hp == 2 and jt == 4 * ic:
                        flush_key(("k23", ic))
                    if hp == 0 and jt == min(4 * ic + 2, 4 * ic + 3):
                        flush_key(("v1", ic))
                    pv, deficit_delta = emit_group(ic, hp, jt, uA, uB)
                    run_pending(keep=2)
                    pull_filler()
                    pending.append(pv)
                    state["deficit"] += deficit_delta
            run_pending(keep=0)
            if ic + 2 < NIC:
                emit_xT(ic + 2)
            for co0 in (0, 2, 4, 6):
                filler.append((1820, ("P", ic), lambda t=ic, c=co0: emit_proj_unit(t, c)))
        for _, _, fn in filler:
            fn()

def _build_program():
    nc = bacc.Bacc("TRN2", target_bir_lowering=False, debug=False, num_devices=N_CORES)
    aps = {
        "x": nc.dram_tensor("x", [T, C], f16, kind="ExternalInput").ap(),
        "wqk0": nc.dram_tensor("wqk0", [C, 256], f16, kind="ExternalInput").ap(),
        "wqk1": nc.dram_tensor("wqk1", [C, 256], f16, kind="ExternalInput").ap(),
        "wqk2": nc.dram_tensor("wqk2", [C, 256], f16, kind="ExternalInput").ap(),
        "wqk3": nc.dram_tensor("wqk3", [C, 256], f16, kind="ExternalInput").ap(),
        "wv": nc.dram_tensor("wv", [C, CL], f16, kind="ExternalInput").ap(),
        "wp": nc.dram_tensor("wp", [CL, C], f16, kind="ExternalInput").ap(),
        "bqk": nc.dram_tensor("bqk", [CB, 128], f32, kind="ExternalInput").ap(),
        "negI": nc.dram_tensor("negI", [128, 128], f16, kind="ExternalInput").ap(),
        "mask2": nc.dram_tensor("mask2", [128, 2, 128], f16, kind="ExternalInput").ap(),
        "outT": nc.dram_tensor("outT", [C, T], f16, kind="ExternalOutput").ap(),
    }
    with tile.TileContext(nc) as tc:
        _emit(tc, aps)
    nc.compile()
    return nc


def get_program():
    if "nc" not in _PROG_CACHE:
        _PROG_CACHE["nc"] = _build_program()
    return _PROG_CACHE["nc"]


def _host_consts():
    r = np.arange(128)[:, None]
    c = np.arange(128)[None, :]
    m = (r > c).astype(np.float16)  # mask-out within a diagonal 128-block
    mask2 = np.ascontiguousarray(np.stack([m, m], axis=1))  # [128, 2, 128]
    negI = np.ascontiguousarray((-60000.0 * np.eye(128)).astype(np.float16))
    return negI, mask2


def make_in_maps(x, W_attn, b_attn, W_proj):
    """Build the 8 per-core input maps. Core index = 2*batch + head_group."""
    negI, mask2 = _host_consts()
    in_maps = []
    for core in range(N_CORES):
        b = core // 2
        g = core % 2
        wq = W_attn[:, g * CL : (g + 1) * CL].astype(np.float16)
        wk = W_attn[:, C + g * CL : C + (g + 1) * CL].astype(np.float16)
        wv = W_attn[:, 2 * C + g * CL : 2 * C + (g + 1) * CL]
        bq = b_attn[g * CL : (g + 1) * CL]
        bk = b_attn[C + g * CL : C + (g + 1) * CL]
        # device co slots: [q01, k01, q23, k23]
        bqk = np.concatenate([bq[0:256], bk[0:256], bq[256:512], bk[256:512]])
        bqk = bqk.reshape(CB, 128)
        in_maps.append(
            {
                "x": np.ascontiguousarray(x[b]).astype(np.float16),
                "wqk0": np.ascontiguousarray(wq[:, 0:256]),
                "wqk1": np.ascontiguousarray(wk[:, 0:256]),
                "wqk2": np.ascontiguousarray(wq[:, 256:512]),
                "wqk3": np.ascontiguousarray(wk[:, 256:512]),
                "wv": np.ascontiguousarray(wv).astype(np.float16),
                "wp": np.ascontiguousarray(W_proj[g * CL : (g + 1) * CL, :]).astype(
                    np.float16
                ),
                "bqk": np.ascontiguousarray(bqk).astype(np.float32),
                "negI": negI,
                "mask2": mask2,
            }
        )
    return in_maps


def run(x, W_attn, b_attn, W_proj, b_proj, trace=False):
    nc = get_program()
    in_maps = make_in_maps(x, W_attn, b_attn, W_proj)
    res = bass_utils.run_bass_kernel_spmd(
        nc, in_maps, core_ids=list(range(N_CORES)), trace=trace
    )
    # combine: out[b] = sum_g outT_{2b+g}^T + (bv_g @ Wp_g summed) + b_proj
    corr = b_proj.astype(np.float64).copy()
    for g in range(2):
        bv_g = b_attn[2 * C + g * CL : 2 * C + (g + 1) * CL]
        corr += bv_g.astype(np.float64) @ W_proj[g * CL : (g + 1) * CL, :].astype(
            np.float64
        )
    out = np.empty((B, T, C), np.float32)
    for b in range(B):
        acc = (
            res.results[2 * b]["outT"].T.astype(np.float64)
            + res.results[2 * b + 1]["outT"].T.astype(np.float64)
            + corr
        )
        out[b] = acc.astype(np.float32)
    return out, res


def kernel(x, W_attn, b_attn, W_proj, b_proj):
    x = np.asarray(x, np.float32)
    W_attn = np.asarray(W_attn, np.float32)
    b_attn = np.asarray(b_attn, np.float32)
    W_proj = np.asarray(W_proj, np.float32)
    b_proj = np.asarray(b_proj, np.float32)
    out, _ = run(x, W_attn, b_attn, W_proj, b_proj)
    return out


# revision 15
# speedup vs baseline: 1.7362x; 1.7362x over previous
"""Causal self-attention (B=4, T=2048, C=1024, H=16) on 8 Trainium2 NeuronCores.

Core index = 2*batch + head_group: each core owns one batch element and 8 of
the 16 heads (tensor-parallel split of c_attn output dim / c_proj input dim).
Each core emits a partial projection out^T [C, T] in fp16; the host sums the
two head-group partials per batch and adds the bias terms.

fp16 datapath (fp32 PSUM accumulation, fp32 softmax denominator).

Schedule (single emission-ordered stream; Tile preserves per-engine order):
  B(tn) units: qkT[co-pair] = W_qk^T x^T (+bias, DVE); v = x @ W_v (ACT copy)
  C groups (ic, hp, jt): head-PAIR processing — heads 2hp (partitions 0:64)
     and 2hp+1 (64:128) issue row-tiled S^T matmuls back-to-back so they run
     CONCURRENTLY on the PE (K=64 each, tile_position (0,0)/(64,0) auto).
     One exp ACT covers both heads, trimmed to [lo:512] on diagonal blocks;
     causal masking via post-exp DVE multiply with a constant tri tile.
     PV matmuls of group g-1 are emitted after S of group g (software
     pipelining) so the PE never waits on the ACT exp.
  B(tn+1) and D projection units are interleaved into the C stream by an
     ACT-vs-PE deficit counter: the C stream alone is ACT-bound ~1.7x, so
     independent full-array matmuls fill the PE and keep HAM at K=8/8.
  D: out^T = W_p^T yT -> fp32 psum -> DVE fp16 copy -> DMA out.
"""

import numpy as np

import concourse.bass as bass
import concourse.mybir as mybir
import concourse.tile as tile
from concourse import bacc, bass_utils

B, T, C, H = 4, 2048, 1024, 16
HD = C // H          # 64 head dim
N_CORES = 8
HG = H // 2          # 8 heads per core
CL = HG * HD         # 512 local width of q/k/v
TT = T // 128        # 16 t-tiles
CB = C // 128        # 8 c-tiles
DB = CL // 128       # 4 local-hd tiles
NIC = T // 512       # i-chunks (4)

f32 = mybir.dt.float32
f16 = mybir.dt.float16

_PROG_CACHE = {}


def _emit(tc, aps):
    nc = tc.nc
    Exp = mybir.ActivationFunctionType.Exp
    Copy = mybir.ActivationFunctionType.Copy

    from contextlib import ExitStack

    with ExitStack() as outer:
        const = outer.enter_context(tc.tile_pool(name="const", bufs=1))
        p_xT = outer.enter_context(tc.tile_pool(name="xT", bufs=1))
        p_w = outer.enter_context(tc.tile_pool(name="wsb", bufs=1))
        p_qkT = outer.enter_context(tc.tile_pool(name="qkT", bufs=1))
        p_v = outer.enter_context(tc.tile_pool(name="vv", bufs=1))
        p_yT = outer.enter_context(tc.tile_pool(name="yT", bufs=1))
        p_pt = outer.enter_context(tc.tile_pool(name="pt", bufs=6))
        p_ot = outer.enter_context(tc.tile_pool(name="ot", bufs=3))
        p_r = outer.enter_context(tc.tile_pool(name="rpool", bufs=4))
        ps = outer.enter_context(tc.tile_pool(name="ps", bufs=3, space="PSUM"))
        ps_u = outer.enter_context(tc.tile_pool(name="psu", bufs=2, space="PSUM"))

        # ---- input DMAs: first-needed first; consts on the gpsimd queue ----
        # device co slots: [q01, k01, q23, k23] pairs -> hp0/hp1 unlock first
        wqk_sb = p_w.tile([128, CB, CB * 128], f16)  # [c-part, cb, co*128+...]
        def emit_wqk_dma(ci):
            nc.scalar.dma_start(
                wqk_sb[:, :, ci * 256 : (ci + 1) * 256],
                aps[f"wqk{ci}"].rearrange("(cb p) n -> p cb n", p=128),
            )
        emit_wqk_dma(0)
        emit_wqk_dma(1)
        xT = p_xT.tile([128, CB, T], f16)

        def emit_xT(tn):
            if tn > 0:  # WAW fence: delays the DMA until DVE reaches this point
                nc.vector.memset(xT[:, 0:1, tn * 512 : tn * 512 + 8], 0.0)
            nc.sync.dma_start_transpose(
                xT[:, :, tn * 512 : (tn + 1) * 512],
                aps["x"][tn * 512 : (tn + 1) * 512, :],
            )

        emit_xT(0)
        wv_sb = p_w.tile([128, CB, CL], f16)
        nc.scalar.dma_start(wv_sb[:], aps["wv"].rearrange("(cb p) n -> p cb n", p=128))
        emit_wqk_dma(2)
        emit_wqk_dma(3)
        wp_sb = p_w.tile([128, DB, C], f16)
        nc.scalar.dma_start(wp_sb[:], aps["wp"].rearrange("(db p) c -> p db c", p=128))
        negI = const.tile([128, 128], f16)  # -60000 * I
        nc.gpsimd.dma_start(negI[:], aps["negI"])
        mask2 = const.tile([128, 2, 128], f16)  # 1 where row > col (mask out)
        nc.gpsimd.dma_start(mask2[:], aps["mask2"])
        bqk = const.tile([128, CB], f32)
        nc.gpsimd.dma_start(bqk[:], aps["bqk"].rearrange("co p -> p co"))
        warm = const.tile([1, 8], f32)
        nc.scalar.activation(warm[0:1, 0:1], bqk[0:1, 0:1], Exp)  # preload exp table

        qkT = {
            (co, tn): p_qkT.tile([128, 512], f16, tag=f"qkT_{co}_{tn}", name=f"qkT_{co}_{tn}")
            for co in range(CB)
            for tn in range(NIC)
        }
        vv = {}
        for jt in range(TT):
            vv[jt] = p_v.tile([128, HG, HD + 1], f16, tag=f"vv_{jt}", name=f"vv_{jt}")
            nc.vector.memset(vv[jt][:, :, HD : HD + 1], 1.0)
        yTn = {tn: p_yT.tile([128, DB, 512], f16, tag=f"yT_{tn}", name=f"yT_{tn}") for tn in range(NIC)}

        # ------------- emission units -------------
        def emit_qk_unit(tn, co0):
            """qkT tiles for co0, co0+1 at i-chunk tn (16 MMs + 2 DVE adds)."""
            g = ps.tile([128, 2, 512], f32, tag="g", name="g")
            for ix in range(2):
                co = co0 + ix
                for cb in range(CB):
                    nc.tensor.matmul(
                        g[:, ix, :],
                        wqk_sb[:, cb, co * 128 : (co + 1) * 128],
                        xT[:, cb, tn * 512 : (tn + 1) * 512],
                        start=(cb == 0),
                        stop=(cb == CB - 1),
                    )
            for ix in range(2):
                co = co0 + ix
                nc.vector.tensor_scalar_add(
                    qkT[(co, tn)][:], g[:, ix, :], bqk[:, co : co + 1]
                )

        def emit_v_unit(tn, u):
            """vv tiles for t-tiles 4*tn+2u, +1 (16 MMs + 2 ACT copies)."""
            g = ps.tile([128, 2, 512], f32, tag="g", name="g")
            for ix in range(2):
                tt = 4 * tn + 2 * u + ix
                for cb in range(CB):
                    nc.tensor.matmul(
                        g[:, ix, :],
                        xT[:, cb, tt * 128 : (tt + 1) * 128],
                        wv_sb[:, cb, :],
                        start=(cb == 0),
                        stop=(cb == CB - 1),
                    )
            for ix in range(2):
                tt = 4 * tn + 2 * u + ix
                nc.vector.tensor_copy(
                    vv[tt][:, :, 0:HD],
                    g[:, ix, :].rearrange("p (h d) -> p h d", d=HD),
                )

        def emit_proj_unit(tn, co0):
            """out^T rows for co0, co0+1 at i-chunk tn (8 MMs + ACT copy + DMA)."""
            g = ps.tile([128, 2, 512], f32, tag="g", name="g")
            for ix in range(2):
                co = co0 + ix
                for db in range(DB):
                    nc.tensor.matmul(
                        g[:, ix, :],
                        wp_sb[:, db, co * 128 : (co + 1) * 128],
                        yTn[tn][:, db, :],
                        start=(db == 0),
                        stop=(db == DB - 1),
                    )
            ot = p_ot.tile([128, 2, 512], f16, tag="ot", name="ot")
            nc.vector.tensor_copy(ot[:], g[:])
            for ix in range(2):
                co = co0 + ix
                nc.sync.dma_start(
                    aps["outT"][co * 128 : (co + 1) * 128, tn * 512 : (tn + 1) * 512],
                    ot[:, ix, :],
                )

        def emit_normalize(hp, ic, u, poff):
            # two PSUM reads release the U accumulator slot; rest runs on SBUF
            usb = p_r.tile([HD, 512], f32, tag="usb", name="usb")
            nc.vector.tensor_copy(usb[:], u[0:HD, :])
            rs = p_r.tile([1, 512], f32, tag="rs", name="rs")
            nc.vector.tensor_copy(rs[:], u[HD : HD + 1, :])
            rr = p_r.tile([1, 512], f32, tag="rr", name="rr")
            nc.vector.reciprocal_approx_fast(rr[:], rs[:])
            rb = p_r.tile([HD, 512], f32, tag="rb", name="rb")
            nc.gpsimd.partition_broadcast(rb[:], rr[0:1, :], channels=HD)
            nc.vector.tensor_mul(yTn[ic][poff : poff + HD, hp, :], usb[:], rb[:])

        CO_Q = (0, 1, 4, 5)
        CO_K = (2, 3, 6, 7)

        def emit_group(ic, hp, jt, uA, uB):
            """S^T for head pair (2hp, 2hp+1) at (jt, ic); returns PV closure."""
            co_q, co_k = CO_Q[hp], CO_K[hp]
            m = jt % 4
            diag = jt // 4 == ic
            lo = 128 * m if diag else 0
            kt = jt // 4
            g = ps.tile([128, 2, 512], f32, tag="g", name="g")
            nc.tensor.matmul(
                g[:, 0, lo:512],
                qkT[(co_k, kt)][0:64, m * 128 : (m + 1) * 128],
                qkT[(co_q, ic)][0:64, lo:512],
                start=True,
                stop=not diag,
                skip_group_check=True,
            )
            nc.tensor.matmul(
                g[:, 1, lo:512],
                qkT[(co_k, kt)][64:128, m * 128 : (m + 1) * 128],
                qkT[(co_q, ic)][64:128, lo:512],
                start=True,
                stop=not diag,
                skip_group_check=True,
            )
            if diag:  # -60000 above the block diagonal -> exp == 0
                nc.tensor.matmul(
                    g[:, 0:2, lo : lo + 128], negI[:], mask2[:],
                    start=False, stop=True, skip_group_check=True,
                )
            pt = p_pt.tile([128, 2, 512], f16, tag="pt", name="pt")
            nc.scalar.activation(
                pt[:, 0:2, lo:512], g[:, 0:2, lo:512], Exp, scale=1.0 / np.sqrt(HD)
            )

            def pv():
                nc.tensor.matmul(
                    uA[:, lo:512],
                    vv[jt][:, 2 * hp, :],
                    pt[:, 0, lo:512],
                    start=(jt == 0),
                    stop=(jt == 4 * ic + 3),
                )
                nc.tensor.matmul(
                    uB[:, lo:512],
                    vv[jt][:, 2 * hp + 1, :],
                    pt[:, 1, lo:512],
                    start=(jt == 0),
                    stop=(jt == 4 * ic + 3),
                )
                if jt == 4 * ic + 3:
                    emit_normalize(hp, ic, uA, 0)
                    emit_normalize(hp, ic, uB, 64)

            w = 512 - lo
            act_ns = (2 * w + 352) / 1.2 + 100
            pe_ns = 3 * w / 2.4 + 120 + (280 if diag else 0)
            return pv, act_ns - pe_ns

        # ------------- the schedule -------------
        # B(0): q01+k01 unlock C(0) hp0/hp1; q23/k23 go through the filler
        emit_qk_unit(0, 0)
        emit_qk_unit(0, 2)
        emit_v_unit(0, 0)
        emit_v_unit(0, 1)
        emit_xT(1)

        filler = []  # (pe_cost_ns, key, fn) in emission-feasible order
        state = {"deficit": 7500.0}
        pending = []  # PV closures, lag 2

        def run_pending(keep=0):
            while len(pending) > keep:
                pending.pop(0)()

        def pull_filler():
            while filler and state["deficit"] >= filler[0][0]:
                pe_cost, _, fn = filler.pop(0)
                fn()
                state["deficit"] -= pe_cost

        def flush_key(key):
            kept = []
            for item in filler:
                if item[1] == key:
                    item[2]()
                    state["deficit"] -= item[0]
                else:
                    kept.append(item)
            filler[:] = kept
            state["deficit"] = max(state["deficit"], -3000.0)

        filler.append((3600, ("q23", 0), lambda: emit_qk_unit(0, 4)))
        filler.append((3600, ("k23", 0), lambda: emit_qk_unit(0, 6)))
        for ic in range(NIC):
            if ic + 1 < NIC:
                tn = ic + 1
                filler.append((3600, ("q01", tn), lambda t=tn: emit_qk_unit(t, 0)))
                filler.append((3600, ("k01", tn), lambda t=tn: emit_qk_unit(t, 2)))
                filler.append((3600, ("q23", tn), lambda t=tn: emit_qk_unit(t, 4)))
                filler.append((3600, ("k23", tn), lambda t=tn: emit_qk_unit(t, 6)))
                filler.append((3600, ("v0", tn), lambda t=tn: emit_v_unit(t, 0)))
                filler.append((3600, ("v1", tn), lambda t=tn: emit_v_unit(t, 1)))
            for hp in range(4):
                if hp == 0:
                    flush_key(("q01", ic))
                if hp == 2:
                    flush_key(("q23", ic))
                uA = ps_u.tile([HD + 1, 512], f32, tag="u", name="uA")
                uB = ps_u.tile([HD + 1, 512], f32, tag="u", name="uB")
                for jt in range(4 * (ic + 1)):
                    if hp == 0 and jt == 4 * ic:
                        flush_key(("k01", ic))
                        flush_key(("v0", ic))
                    if hp == 2 and jt == 4 * ic:
                        flush_key(("k23", ic))
                    if hp == 0 and jt == min(4 * ic + 2, 4 * ic + 3):
                        flush_key(("v1", ic))
                    pv, deficit_delta = emit_group(ic, hp, jt, uA, uB)
                    run_pending(keep=2)
                    pull_filler()
                    pending.append(pv)
                    state["deficit"] += deficit_delta
            run_pending(keep=0)
            if ic + 2 < NIC:
                emit_xT(ic + 2)
            for co0 in (0, 2, 4, 6):
                filler.append((1820, ("P", ic), lambda t=ic, c=co0: emit_proj_unit(t, c)))
        for _, _, fn in filler:
            fn()

def _build_program():
    nc = bacc.Bacc("TRN2", target_bir_lowering=False, debug=False, num_devices=N_CORES)
    aps = {
        "x": nc.dram_tensor("x", [T, C], f16, kind="ExternalInput").ap(),
        "wqk0": nc.dram_tensor("wqk0", [C, 256], f16, kind="ExternalInput").ap(),
        "wqk1": nc.dram_tensor("wqk1", [C, 256], f16, kind="ExternalInput").ap(),
        "wqk2": nc.dram_tensor("wqk2", [C, 256], f16, kind="ExternalInput").ap(),
        "wqk3": nc.dram_tensor("wqk3", [C, 256], f16, kind="ExternalInput").ap(),
        "wv": nc.dram_tensor("wv", [C, CL], f16, kind="ExternalInput").ap(),
        "wp": nc.dram_tensor("wp", [CL, C], f16, kind="ExternalInput").ap(),
        "bqk": nc.dram_tensor("bqk", [CB, 128], f32, kind="ExternalInput").ap(),
        "negI": nc.dram_tensor("negI", [128, 128], f16, kind="ExternalInput").ap(),
        "mask2": nc.dram_tensor("mask2", [128, 2, 128], f16, kind="ExternalInput").ap(),
        "outT": nc.dram_tensor("outT", [C, T], f16, kind="ExternalOutput").ap(),
    }
    with tile.TileContext(nc) as tc:
        _emit(tc, aps)
    nc.compile()
    return nc


def get_program():
    if "nc" not in _PROG_CACHE:
        _PROG_CACHE["nc"] = _build_program()
    return _PROG_CACHE["nc"]


def _host_consts():
    r = np.arange(128)[:, None]
    c = np.arange(128)[None, :]
    m = (r > c).astype(np.float16)  # mask-out within a diagonal 128-block
    mask2 = np.ascontiguousarray(np.stack([m, m], axis=1))  # [128, 2, 128]
    negI = np.ascontiguousarray((-60000.0 * np.eye(128)).astype(np.float16))
    return negI, mask2


def make_in_maps(x, W_attn, b_attn, W_proj):
    """Build the 8 per-core input maps. Core index = 2*batch + head_group."""
    negI, mask2 = _host_consts()
    in_maps = []
    for core in range(N_CORES):
        b = core // 2
        g = core % 2
        wq = W_attn[:, g * CL : (g + 1) * CL].astype(np.float16)
        wk = W_attn[:, C + g * CL : C + (g + 1) * CL].astype(np.float16)
        wv = W_attn[:, 2 * C + g * CL : 2 * C + (g + 1) * CL]
        bq = b_attn[g * CL : (g + 1) * CL]
        bk = b_attn[C + g * CL : C + (g + 1) * CL]
        # device co slots: [q01, k01, q23, k23]
        bqk = np.concatenate([bq[0:256], bk[0:256], bq[256:512], bk[256:512]])
        bqk = bqk.reshape(CB, 128)
        in_maps.append(
            {
                "x": np.ascontiguousarray(x[b]).astype(np.float16),
                "wqk0": np.ascontiguousarray(wq[:, 0:256]),
                "wqk1": np.ascontiguousarray(wk[:, 0:256]),
                "wqk2": np.ascontiguousarray(wq[:, 256:512]),
                "wqk3": np.ascontiguousarray(wk[:, 256:512]),
                "wv": np.ascontiguousarray(wv).astype(np.float16),
                "wp": np.ascontiguousarray(W_proj[g * CL : (g + 1) * CL, :]).astype(
                    np.float16
                ),
                "bqk": np.ascontiguousarray(bqk).astype(np.float32),
                "negI": negI,
                "mask2": mask2,
            }
        )
    return in_maps


def run(x, W_attn, b_attn, W_proj, b_proj, trace=False):
    nc = get_program()
    in_maps = make_in_maps(x, W_attn, b_attn, W_proj)
    res = bass_utils.run_bass_kernel_spmd(
        nc, in_maps, core_ids=list(range(N_CORES)), trace=trace
    )
    # combine: out[b] = sum_g outT_{2b+g}^T + (bv_g @ Wp_g summed) + b_proj
    corr = b_proj.astype(np.float64).copy()
    for g in range(2):
        bv_g = b_attn[2 * C + g * CL : 2 * C + (g + 1) * CL]
        corr += bv_g.astype(np.float64) @ W_proj[g * CL : (g + 1) * CL, :].astype(
            np.float64
        )
    out = np.empty((B, T, C), np.float32)
    for b in range(B):
        acc = (
            res.results[2 * b]["outT"].T.astype(np.float64)
            + res.results[2 * b + 1]["outT"].T.astype(np.float64)
            + corr
        )
        out[b] = acc.astype(np.float32)
    return out, res


def kernel(x, W_attn, b_attn, W_proj, b_proj):
    x = np.asarray(x, np.float32)
    W_attn = np.asarray(W_attn, np.float32)
    b_attn = np.asarray(b_attn, np.float32)
    W_proj = np.asarray(W_proj, np.float32)
    b_proj = np.asarray(b_proj, np.float32)
    out, _ = run(x, W_attn, b_attn, W_proj, b_proj)
    return out
